# revision 21
# baseline (speedup 1.0000x reference)
"""Single-head causal attention (B=4, T=2048, C=1024, H=64) on 8 NeuronCores.

Sharding: 8 cores = 4 batches x 2 interleaved halves. Core (b, h) computes
query blocks of 512 rows: h=0 -> rows [0:512] and [1024:1536]; h=1 -> rows
[512:1024] and [1536:2048]. This balances causal work while keeping ONE SPMD
program: all per-core differences enter through input DATA.

Causality, with zero per-chunk instructions:
  - the score matmuls contract over K=66: rows 0:64 are the head dim, rows
    64:65 of the key operand hold per-(block, chunk) biases (0 or -1e30/scale)
    and the query operand holds block-selector rows (1/0). Acausal chunks thus
    come out of the matmul pre-biased to -1e30 and exp() kills them for free.
  - diagonal (partially causal) chunks are masked post-exp by gpsimd
    affine_select (no mask tile, no DMA).

Layout: scores are computed transposed (scoresT[tk, tq]) so softmax sums come
from the PV matmul itself: V is augmented with a ones column -> PV psum row 64
is the denominator.

v3 pipeline (from perfetto evidence): TRN2's PE clock ramps 0.65 -> 1.2 ->
2.4 GHz with 3us of *continuous* execution and any idle gap resets it, so the
whole kernel is laid out as one dense PE stream: garbage-operand warm-up
matmuls spin the PE from t~6us while the first DMAs land; x arrives as 256KB
quarter-chunks in two need-ordered HWDGE streams; projection matmul steps are
injected as PE filler between attention PV matmuls (which are ACT-paced) so
neither engine ever waits long; the block-0 epilogue transposes ride inside
the last attention phase and every epilogue divides + stores per-128-row
slice to shorten the drain. Epilogue and output are bf16.
"""

import numpy as np
import ml_dtypes

import concourse.bass as bass
from concourse import bacc
import concourse.mybir as mybir
import concourse.tile as tile
from concourse.bass_utils import run_bass_kernel_spmd

B, T, C, H = 4, 2048, 1024, 64
P = 128
TQ = 512                 # query block width
NBLK = 2                 # query blocks per core
NQ = NBLK * TQ           # 1024 query rows per core
SCHED = (4, 12)          # full-phase k-chunks per block (compile-time max)
NDIAG = TQ // P          # 4 diagonal chunks per block
KFULL = SCHED[-1] * P    # 1536 k columns needed for full phase
NKCH = KFULL // TQ       # 3 xk column chunks
CCH = C // P             # 8 contraction chunks
NV = NDIAG * NBLK + SCHED[-1]   # 8 diag + 12 full v blocks of 128 rows
SCALE = float(C) ** -0.5
BIGNEG = -1e30 / SCALE   # lands as -1e30 after the exp scale

F32 = mybir.dt.float32
BF16 = mybir.dt.bfloat16
NPBF = ml_dtypes.bfloat16

_CACHE = {}


def build():
    nc = bacc.Bacc()
    xq_d = nc.declare_dram_parameter("xq", [NBLK, P, CCH * TQ], BF16, isOutput=False)
    xk_d = nc.declare_dram_parameter("xk", [NKCH, P, CCH * TQ], BF16, isOutput=False)
    # stair | wqv | wk | wkv | identb packed into one tensor: one trigger,
    # 7KB descriptor lines
    wpack_d = nc.declare_dram_parameter("wpack", [P, 3584], BF16, isOutput=False)
    aug_d = nc.declare_dram_parameter("aug", [2, KFULL], BF16, isOutput=False)
    out_d = nc.declare_dram_parameter("out", [P, NBLK * NDIAG * H], BF16, isOutput=True)

    EXPF = mybir.ActivationFunctionType.Exp

    with tile.TileContext(nc) as tc:
        with (
            tc.tile_pool(name="big", bufs=1) as big,
            tc.tile_pool(name="work", bufs=6) as work,
            tc.tile_pool(name="epi", bufs=6) as epi,
            tc.tile_pool(name="psp", bufs=2, space="PSUM") as psp,
            tc.tile_pool(name="pss", bufs=2, space="PSUM") as pss,
            tc.tile_pool(name="pspv", bufs=2, space="PSUM") as pspv,
            tc.tile_pool(name="pstr", bufs=2, space="PSUM") as pstr,
        ):
            # ---- DMA triggers: need-ordered, few and fat (each trigger
            # costs ~600ns of sequencer time and ring slots scale with
            # descriptor count, so fewer/bigger transfers win).
            # per-core ktb bias rows: tiny SWDGE transfer, fired first
            ktb = big.tile([66, KFULL], BF16)
            nc.gpsimd.dma_start(out=ktb[64:66, :], in_=aug_d[:])
            # warm-up operand: one cheap DVE memset, then the PE spins on it
            wgl = big.tile([P, 256], BF16)
            nc.vector.memset(wgl[:], 0.0)
            wpack = big.tile([P, 3584], BF16)
            nc.sync.dma_start(out=wpack[:], in_=wpack_d[:])
            # weight/const views into wpack
            wqv = lambda cc: wpack[:, 896 + cc * 128: 896 + (cc + 1) * 128]
            wkf = lambda cc: wpack[:, 1920 + cc * 64: 1920 + (cc + 1) * 64]
            wkv = lambda cc: wpack[:, 2432 + cc * 128: 2432 + (cc + 1) * 128]
            IDB = 3456
            idb_hi = wpack[64:128, IDB + 64:IDB + 128]
            idb_65 = wpack[0:H + 1, IDB:IDB + H + 1]
            # x stream: xq0 in halves (fine-grained start), the rest whole
            xq0h = []
            for hh in range(2):
                t = big.tile([P, 4, TQ], BF16, tag=f"xq0h{hh}")
                [nc.scalar, nc.sync][hh].dma_start(
                    out=t[:],
                    in_=xq_d[0][:, bass.ts(hh, 4 * TQ)].rearrange(
                        "p (c t) -> p c t", c=4))
                xq0h.append(t)
            xq1t = big.tile([P, CCH, TQ], BF16, tag="xq1")
            nc.scalar.dma_start(
                out=xq1t[:], in_=xq_d[1].rearrange("p (c t) -> p c t", c=CCH))
            xkt = []
            for i in range(NKCH):
                t = big.tile([P, CCH, TQ], BF16, tag=f"xk{i}")
                [nc.sync, nc.scalar, nc.sync][i].dma_start(
                    out=t[:], in_=xk_d[i].rearrange("p (c t) -> p c t", c=CCH))
                xkt.append(t)

            def xq_ap(blk, cc):
                if blk == 0:
                    return xq0h[cc // 4][:, cc % 4, :]
                return xq1t[:, cc, :]

            # core-uniform aug rows generated on gpsimd (DVE is needed for
            # the warm-up memset; gpsimd is otherwise idle):
            # qb rows 64:65 select block 0/1; kdb bias rows are zero
            qb = big.tile([66, NQ], BF16)
            nc.gpsimd.memset(qb[64:66, :], 0.0)
            nc.gpsimd.memset(qb[64:66, TQ:NQ], 1.0)
            nc.gpsimd.memset(qb[64:65, TQ:NQ], 0.0)
            nc.gpsimd.memset(qb[64:65, 0:TQ], 1.0)
            kdb = big.tile([66, NQ], BF16)
            nc.gpsimd.memset(kdb[64:66, :], 0.0)

            # ---- v_aug ones column + PE p-state warm-up ----
            vaug = big.tile([P, NV, H + 1], BF16)
            nc.vector.memset(vaug[:, :, H], 1.0)
            for w in range(26):
                wu = pss.tile([P, 256], F32, tag="s")
                nc.tensor.matmul(wu[:], wgl[:, 0:128], wgl[:], start=True, stop=True)

            vdh = big.tile([P, NQ], BF16)      # v of own q rows, partitions 64:128
            vfu = big.tile([P, KFULL], BF16)   # v of prefix rows, partitions 64:128

            # ---- projection steps (closures; used inline or as PE filler) --
            def proj_xq_steps(blk):
                st = {"qv": None, "kd": None}
                sl = bass.ts(blk, TQ)

                def qv_step(hh, jj):
                    def go():
                        if st["qv"] is None:
                            tqv = psp.tile([P, TQ], F32, tag="proj")
                            st["qv"] = tqv
                        for j in range(2):
                            cc = 4 * hh + 2 * jj + j
                            nc.tensor.matmul(st["qv"][:], wqv(cc),
                                             xq_ap(blk, cc),
                                             start=(cc == 0), stop=(cc == CCH - 1))
                        if hh == 1 and jj == 1:
                            o = blk * TQ
                            for j in range(NDIAG):
                                c = bass.ts(j, P)
                                nc.vector.tensor_copy(
                                    vdh[64:128, o + j * P:o + (j + 1) * P],
                                    st["qv"][64:128, c])
                            for hq in range(2):
                                c = bass.ts(hq, 256)
                                nc.vector.tensor_copy(
                                    qb[0:64, o + hq * 256:o + (hq + 1) * 256],
                                    st["qv"][0:64, c])
                    return go

                def kd_step(hh, jj):
                    def go():
                        if st["kd"] is None:
                            tkd = psp.tile([P, TQ], F32, tag="proj")
                            st["kd"] = tkd
                        for j in range(2):
                            cc = 4 * hh + 2 * jj + j
                            nc.tensor.matmul(st["kd"][0:64, :], wkf(cc),
                                             xq_ap(blk, cc),
                                             start=(cc == 0), stop=(cc == CCH - 1))
                        if hh == 1 and jj == 1:
                            o = blk * TQ
                            for j in range(NDIAG):
                                c = bass.ts(j, P)
                                nc.vector.tensor_copy(
                                    kdb[0:64, o + j * P:o + (j + 1) * P],
                                    st["kd"][0:64, c])
                    return go

                # kd on half-0 fills the wait for half-1 of xq
                return [qv_step(0, 0), qv_step(0, 1), kd_step(0, 0), kd_step(0, 1),
                        qv_step(1, 0), qv_step(1, 1), kd_step(1, 0), kd_step(1, 1)]

            def proj_xk_steps(seg):
                st = {"kv": None}
                sl = bass.ts(seg, TQ)

                def kv_step(hh, jj):   # two cc per step
                    def go():
                        if st["kv"] is None:
                            tkv = psp.tile([P, TQ], F32, tag="proj")
                            st["kv"] = tkv
                        for j in range(2):
                            cc = 4 * hh + 2 * jj + j
                            nc.tensor.matmul(st["kv"][:], wkv(cc),
                                             xkt[seg][:, cc, :],
                                             start=(cc == 0), stop=(cc == CCH - 1))
                        if hh == 1 and jj == 1:
                            o = seg * TQ
                            for j in range(NDIAG):
                                c = bass.ts(j, P)
                                nc.vector.tensor_copy(
                                    vfu[64:128, o + j * P:o + (j + 1) * P],
                                    st["kv"][64:128, c])
                                nc.vector.tensor_copy(
                                    ktb[0:64, o + j * P:o + (j + 1) * P],
                                    st["kv"][0:64, c])
                    return go

                return [kv_step(hh, jj) for hh in range(2) for jj in range(2)]

            def make_vaug(slot, src_upper, col0):
                tp = pstr.tile([P, H], BF16, tag="tr")
                nc.tensor.transpose(tp[:], src_upper[64:128, col0:col0 + P],
                                    idb_hi)
                nc.vector.tensor_copy(vaug[:, slot, 0:H], tp[:])

            pvs_ps = [None, None]

            def attn_phase(chunks, new_slots, pipe=4, fillers=None):
                """chunks: (blk, kind, c, start, stop). new_slots upfront;
                PVs trail scores by `pipe`; each PV is followed by one filler
                closure (projection work) to keep the PE dense while ACT
                computes the next exp."""
                fillers = list(fillers or [])
                for slot, src, col0 in new_slots:
                    make_vaug(slot, src, col0)
                es = []

                def scores(i):
                    blk, kind, c, _, _ = chunks[i]
                    if kind == "d":
                        slot = blk * NDIAG + c
                        lhsT = kdb[:, blk * TQ + c * P: blk * TQ + (c + 1) * P]
                    else:
                        slot = NBLK * NDIAG + c
                        lhsT = ktb[:, bass.ts(c, P)]
                    s = pss.tile([P, TQ], F32, tag="s")
                    nc.tensor.matmul(s[:], lhsT, qb[0:66, bass.ts(blk, TQ)],
                                     start=True, stop=True)
                    e = work.tile([P, TQ], BF16, tag="e")
                    nc.scalar.activation(e[:], s[:], EXPF, scale=SCALE)
                    if kind == "d":
                        off = 384 - 128 * c
                        nc.vector.tensor_mul(e[:], e[:], wpack[:, off:off + TQ])
                    es.append((e, slot))

                def pv(i):
                    blk, kind, c, st_, sp = chunks[i]
                    e, slot = es[i]
                    nc.tensor.matmul(pvs_ps[blk][0:H + 1, :], vaug[:, slot, :],
                                     e[:], start=st_, stop=sp)
                    if fillers:
                        fillers.pop(0)()

                nxt = 0
                for i in range(len(chunks)):
                    scores(i)
                    if i >= pipe - 1:
                        pv(nxt)
                        nxt += 1
                while nxt < len(chunks):
                    pv(nxt)
                    nxt += 1
                for f in fillers:
                    f()

            def epilogue_copies(blk):
                pvs = epi.tile([H + 1, TQ], BF16, tag=f"pvs{blk}")
                for j in range(NDIAG):
                    c = bass.ts(j, P)
                    nc.vector.tensor_copy(pvs[:, c], pvs_ps[blk][0:H + 1, c])
                return pvs

            def epi_tr_step(blk, pvs, j):
                def go():
                    ot = pstr.tile([P, H + 1], BF16, tag="tr")
                    nc.tensor.transpose(ot[:], pvs[:, bass.ts(j, P)],
                                        idb_65)
                    r = epi.tile([P, 1], F32, tag="r")
                    nc.vector.reciprocal(r[:], ot[:, H:H + 1])
                    ob = bass.ts(blk * NDIAG + j, H)
                    obt = epi.tile([P, H], BF16, tag="ob")
                    nc.vector.tensor_scalar_mul(obt[:], ot[:, 0:H], r[:])
                    nc.sync.dma_start(out=out_d[:, ob], in_=obt[:])
                return go

            # ---- master schedule ----
            diag = lambda blk: [(blk, "d", c, c == 0, False) for c in range(NDIAG)]
            S1 = proj_xq_steps(1)
            S2 = proj_xk_steps(0)
            S3 = proj_xk_steps(1)
            S4 = proj_xk_steps(2)

            for step in proj_xq_steps(0):
                step()
            pv0 = pspv.tile([H + 1, TQ], F32, tag="pv")
            pvs_ps[0] = pv0
            attn_phase(diag(0), [(d, vdh, d * P) for d in range(NDIAG)],
                       fillers=S1[:4])
            for step in S1[4:]:
                step()
            pv1 = pspv.tile([H + 1, TQ], F32, tag="pv")
            pvs_ps[1] = pv1
            attn_phase(diag(1), [(NDIAG + d, vdh, TQ + d * P) for d in range(NDIAG)],
                       fillers=S2)

            phA = ([(0, "f", c, False, c == 3) for c in range(4)] +
                   [(1, "f", c, False, False) for c in range(4)])
            attn_phase(phA, [(NBLK * NDIAG + c, vfu, c * P) for c in range(4)],
                       fillers=S3)
            pvs0 = epilogue_copies(0)

            attn_phase([(1, "f", c, False, False) for c in range(4, 8)],
                       [(NBLK * NDIAG + c, vfu, c * P) for c in range(4, 8)],
                       fillers=S4)

            attn_phase([(1, "f", c, False, c == 11) for c in range(8, 12)],
                       [(NBLK * NDIAG + c, vfu, c * P) for c in range(8, 12)],
                       fillers=[epi_tr_step(0, pvs0, j) for j in range(NDIAG)])

            pvs1 = epilogue_copies(1)
            for j in range(NDIAG):
                epi_tr_step(1, pvs1, j)()
    nc.compile()
    return nc


def _pack_x(xT, cols):
    # xT: [C, T] fp32 -> [P, CCH*W] bf16 in SBUF layout
    a = xT[:, cols]                                   # [C, W]
    a = a.reshape(CCH, P, -1).transpose(1, 0, 2)      # [P, CCH, W]
    return np.ascontiguousarray(a.reshape(P, -1)).astype(NPBF)


def _pack_w(w):
    # w: [C, width] -> [P, CCH*width]
    a = w.reshape(CCH, P, -1).transpose(1, 0, 2)
    return np.ascontiguousarray(a.reshape(P, -1)).astype(NPBF)


def _host_inputs(x, Wk, Wq, Wv):
    ii = np.arange(P)
    stair = (np.arange(896)[None, :] >= ii[:, None] + 384).astype(NPBF)
    wpack = np.concatenate([
        stair,
        _pack_w(np.concatenate([Wq, Wv], axis=1)),
        _pack_w(Wk),
        _pack_w(np.concatenate([Wk, Wv], axis=1)),
        np.eye(P, dtype=NPBF),
    ], axis=1)
    assert wpack.shape == (P, 3584)
    in_maps = []
    for b in range(B):
        xT = np.ascontiguousarray(x[b].T.astype(np.float32))  # [C, T]
        for h in range(2):
            q0s = (0, 1024) if h == 0 else (512, 1536)
            xq = np.stack([_pack_x(xT, slice(q0, q0 + TQ)) for q0 in q0s])
            xk = np.stack([_pack_x(xT, slice(i * TQ, (i + 1) * TQ))
                           for i in range(NKCH)])
            # ktb bias rows: row blk, col t = 0 if chunk t//128 is a (strictly
            # pre-diagonal) causal chunk for this core's block blk, else BIGNEG
            kaug = np.full((2, KFULL), BIGNEG, np.float32)
            for blk, q0 in enumerate(q0s):
                kaug[blk, :q0] = 0.0
            aug = kaug.astype(NPBF)
            in_maps.append(dict(xq=xq, xk=xk, wpack=wpack, aug=aug))
    return in_maps


def kernel(x, Wk, Wq, Wv, trace=False):
    x = np.asarray(x, np.float32)
    in_maps = _host_inputs(x, np.asarray(Wk, np.float32),
                           np.asarray(Wq, np.float32), np.asarray(Wv, np.float32))
    if "nc" not in _CACHE:
        _CACHE["nc"] = build()
    nc = _CACHE["nc"]
    res = run_bass_kernel_spmd(nc, in_maps, list(range(8)), trace=trace)
    out = np.empty((B, T, H), np.float32)
    for b in range(B):
        for h in range(2):
            o = res.results[b * 2 + h]["out"]  # [P, NBLK*NDIAG*H] bf16
            o = np.asarray(o).astype(np.float32).reshape(P, NBLK, NDIAG, H)
            q0s = (0, 1024) if h == 0 else (512, 1536)
            for blk, q0 in enumerate(q0s):
                # row q0 + j*128 + p  <-  o[p, blk, j, :]
                out[b, q0:q0 + TQ] = o[:, blk].transpose(1, 0, 2).reshape(TQ, H)
    kernel.last_exec_time_ns = res.exec_time_ns
    kernel.last_results = res
    return out


# revision 24
# speedup vs baseline: 1.2037x; 1.2037x over previous
"""Single-head causal attention (B=4, T=2048, C=1024, H=64) on 8 NeuronCores.

Sharding: 8 cores = 4 batches x 2 interleaved halves. Core (b, h) computes
query blocks of 512 rows: h=0 -> rows [0:512] and [1024:1536]; h=1 -> rows
[512:1024] and [1536:2048]. This balances causal work while keeping ONE SPMD
program: all per-core differences enter through input DATA.

Causality, with zero per-chunk instructions:
  - the score matmuls contract over K=66: rows 0:64 are the head dim, rows
    64:65 of the key operand hold per-(block, chunk) biases (0 or -1e30/scale)
    and the query operand holds block-selector rows (1/0). Acausal chunks thus
    come out of the matmul pre-biased to -1e30 and exp() kills them for free.
  - diagonal (partially causal) chunks are masked post-exp by gpsimd
    affine_select (no mask tile, no DMA).

Layout: scores are computed transposed (scoresT[tk, tq]) so softmax sums come
from the PV matmul itself: V is augmented with a ones column -> PV psum row 64
is the denominator.

v3 pipeline (from perfetto evidence): TRN2's PE clock ramps 0.65 -> 1.2 ->
2.4 GHz with 3us of *continuous* execution and any idle gap resets it, so the
whole kernel is laid out as one dense PE stream: garbage-operand warm-up
matmuls spin the PE from t~6us while the first DMAs land; x arrives as 256KB
quarter-chunks in two need-ordered HWDGE streams; projection matmul steps are
injected as PE filler between attention PV matmuls (which are ACT-paced) so
neither engine ever waits long; the block-0 epilogue transposes ride inside
the last attention phase and every epilogue divides + stores per-128-row
slice to shorten the drain. Epilogue and output are bf16.
"""

import numpy as np
import ml_dtypes

import concourse.bass as bass
from concourse import bacc
import concourse.mybir as mybir
import concourse.tile as tile
from concourse.bass_utils import run_bass_kernel_spmd

B, T, C, H = 4, 2048, 1024, 64
P = 128
TQ = 512                 # query block width
NBLK = 2                 # query blocks per core
NQ = NBLK * TQ           # 1024 query rows per core
SCHED = (4, 12)          # full-phase k-chunks per block (compile-time max)
NDIAG = TQ // P          # 4 diagonal chunks per block
KFULL = SCHED[-1] * P    # 1536 k columns needed for full phase
NKCH = KFULL // TQ       # 3 xk column chunks
CCH = C // P             # 8 contraction chunks
NV = NDIAG * NBLK + SCHED[-1]   # 8 diag + 12 full v blocks of 128 rows
SCALE = float(C) ** -0.5
BIGNEG = -1e30 / SCALE   # lands as -1e30 after the exp scale

F32 = mybir.dt.float32
BF16 = mybir.dt.bfloat16
NPBF = ml_dtypes.bfloat16

_CACHE = {}


def build():
    nc = bacc.Bacc()
    xq_d = nc.declare_dram_parameter("xq", [NBLK, P, CCH * TQ], BF16, isOutput=False)
    xk_d = nc.declare_dram_parameter("xk", [NKCH, P, CCH * TQ], BF16, isOutput=False)
    # wqv | wk first (gates the first projection); stair | wkv | identb later
    wqk_d = nc.declare_dram_parameter("wqk", [P, 1536], BF16, isOutput=False)
    wrest_d = nc.declare_dram_parameter("wrest", [P, 2048], BF16, isOutput=False)
    aug_d = nc.declare_dram_parameter("aug", [2, KFULL], BF16, isOutput=False)
    out_d = nc.declare_dram_parameter("out", [P, NBLK * NDIAG * H], BF16, isOutput=True)

    EXPF = mybir.ActivationFunctionType.Exp

    with tile.TileContext(nc) as tc:
        with (
            tc.tile_pool(name="big", bufs=1) as big,
            tc.tile_pool(name="work", bufs=6) as work,
            tc.tile_pool(name="epi", bufs=6) as epi,
            tc.tile_pool(name="psp", bufs=2, space="PSUM") as psp,
            tc.tile_pool(name="pss", bufs=2, space="PSUM") as pss,
            tc.tile_pool(name="pspv", bufs=2, space="PSUM") as pspv,
            tc.tile_pool(name="pstr", bufs=2, space="PSUM") as pstr,
        ):
            # ---- DMA triggers: need-ordered, few and fat (each trigger
            # costs ~600ns of sequencer time and ring slots scale with
            # descriptor count, so fewer/bigger transfers win).
            # per-core ktb bias rows: tiny SWDGE transfer, fired first
            ktb = big.tile([66, KFULL], BF16)
            nc.gpsimd.dma_start(out=ktb[64:66, :], in_=aug_d[:])
            # warm-up operand: one cheap DVE memset, then the PE spins on it
            wgl = big.tile([P, 256], BF16)
            nc.vector.memset(wgl[:], 0.0)
            wqk = big.tile([P, 1536], BF16)
            nc.sync.dma_start(out=wqk[:], in_=wqk_d[:])
            wqv = lambda cc: wqk[:, cc * 128: (cc + 1) * 128]
            wkf = lambda cc: wqk[:, 1024 + cc * 64: 1024 + (cc + 1) * 64]
            # x stream: xq0 in halves (fine-grained start), the rest whole
            xq0h = []
            for hh in range(2):
                t = big.tile([P, 4, TQ], BF16, tag=f"xq0h{hh}")
                [nc.scalar, nc.sync][hh].dma_start(
                    out=t[:],
                    in_=xq_d[0][:, bass.ts(hh, 4 * TQ)].rearrange(
                        "p (c t) -> p c t", c=4))
                xq0h.append(t)
            xq1t = big.tile([P, CCH, TQ], BF16, tag="xq1")
            nc.scalar.dma_start(
                out=xq1t[:], in_=xq_d[1].rearrange("p (c t) -> p c t", c=CCH))
            wrest = big.tile([P, 2048], BF16)
            nc.sync.dma_start(out=wrest[:], in_=wrest_d[:])
            wkv = lambda cc: wrest[:, 896 + cc * 128: 896 + (cc + 1) * 128]
            IDB = 1920
            idb_hi = wrest[64:128, IDB + 64:IDB + 128]
            idb_65 = wrest[0:H + 1, IDB:IDB + H + 1]
            xkt = []
            for i in range(NKCH):
                t = big.tile([P, CCH, TQ], BF16, tag=f"xk{i}")
                [nc.sync, nc.scalar, nc.sync][i].dma_start(
                    out=t[:], in_=xk_d[i].rearrange("p (c t) -> p c t", c=CCH))
                xkt.append(t)

            def xq_ap(blk, cc):
                if blk == 0:
                    return xq0h[cc // 4][:, cc % 4, :]
                return xq1t[:, cc, :]

            # core-uniform aug rows generated on gpsimd (DVE is needed for
            # the warm-up memset; gpsimd is otherwise idle):
            # qb rows 64:65 select block 0/1; kdb bias rows are zero
            qb = big.tile([66, NQ], BF16)
            nc.gpsimd.memset(qb[64:66, :], 0.0)
            nc.gpsimd.memset(qb[64:66, TQ:NQ], 1.0)
            nc.gpsimd.memset(qb[64:65, TQ:NQ], 0.0)
            nc.gpsimd.memset(qb[64:65, 0:TQ], 1.0)
            kdb = big.tile([66, NQ], BF16)
            nc.gpsimd.memset(kdb[64:66, :], 0.0)

            # ---- v_aug ones column + PE p-state warm-up ----
            vaug = big.tile([P, NV, H + 1], BF16)
            nc.vector.memset(vaug[:, :, H], 1.0)
            for w in range(34):
                wu = pss.tile([P, 256], F32, tag="s")
                nc.tensor.matmul(wu[:], wgl[:, 0:128], wgl[:], start=True, stop=True)

            vdh = big.tile([P, NQ], BF16)      # v of own q rows, partitions 64:128
            vfu = big.tile([P, KFULL], BF16)   # v of prefix rows, partitions 64:128

            # ---- projection steps (closures; used inline or as PE filler) --
            def proj_xq_steps(blk):
                st = {"qv": None, "kd": None}
                sl = bass.ts(blk, TQ)

                def qv_step(hh, jj):
                    def go():
                        if st["qv"] is None:
                            tqv = psp.tile([P, TQ], F32, tag="proj")
                            st["qv"] = tqv
                        for j in range(2):
                            cc = 4 * hh + 2 * jj + j
                            nc.tensor.matmul(st["qv"][:], wqv(cc),
                                             xq_ap(blk, cc),
                                             start=(cc == 0), stop=(cc == CCH - 1))
                        if hh == 1 and jj == 1:
                            sl = bass.ts(blk, TQ)
                            nc.vector.tensor_copy(qb[0:64, sl], st["qv"][0:64, :])
                            nc.vector.tensor_copy(vdh[64:128, sl], st["qv"][64:128, :])
                    return go

                def kd_step(hh, jj):
                    def go():
                        if st["kd"] is None:
                            tkd = psp.tile([P, TQ], F32, tag="proj")
                            st["kd"] = tkd
                        for j in range(2):
                            cc = 4 * hh + 2 * jj + j
                            nc.tensor.matmul(st["kd"][0:64, :], wkf(cc),
                                             xq_ap(blk, cc),
                                             start=(cc == 0), stop=(cc == CCH - 1))
                        if hh == 1 and jj == 1:
                            sl = bass.ts(blk, TQ)
                            nc.vector.tensor_copy(kdb[0:64, sl], st["kd"][0:64, :])
                    return go

                # kd finishes before qv so its copy overlaps the qv matmuls
                return [qv_step(0, 0), qv_step(0, 1), kd_step(0, 0), kd_step(0, 1),
                        kd_step(1, 0), kd_step(1, 1), qv_step(1, 0), qv_step(1, 1)]

            def proj_xk_steps(seg):
                st = {"kv": None}
                sl = bass.ts(seg, TQ)

                def kv_step(hh, jj):   # two cc per step
                    def go():
                        if st["kv"] is None:
                            tkv = psp.tile([P, TQ], F32, tag="proj")
                            st["kv"] = tkv
                        for j in range(2):
                            cc = 4 * hh + 2 * jj + j
                            nc.tensor.matmul(st["kv"][:], wkv(cc),
                                             xkt[seg][:, cc, :],
                                             start=(cc == 0), stop=(cc == CCH - 1))
                        if hh == 1 and jj == 1:
                            sl = bass.ts(seg, TQ)
                            nc.vector.tensor_copy(ktb[0:64, sl], st["kv"][0:64, :])
                            nc.vector.tensor_copy(vfu[64:128, sl], st["kv"][64:128, :])
                    return go

                return [kv_step(hh, jj) for hh in range(2) for jj in range(2)]

            def make_vaug(slot, src_upper, col0):
                tp = pstr.tile([P, H], BF16, tag="tr")
                nc.tensor.transpose(tp[:], src_upper[64:128, col0:col0 + P],
                                    idb_hi)
                nc.vector.tensor_copy(vaug[:, slot, 0:H], tp[:])

            pvs_ps = [None, None]

            def attn_phase(chunks, new_slots, pipe=4, fillers=None):
                """chunks: (blk, kind, c, start, stop). new_slots upfront;
                PVs trail scores by `pipe`; each PV is followed by one filler
                closure (projection work) to keep the PE dense while ACT
                computes the next exp."""
                fillers = list(fillers or [])
                vts = list(new_slots)
                es = []

                def scores(i):
                    blk, kind, c, _, _ = chunks[i]
                    if kind == "d":
                        slot = blk * NDIAG + c
                        lhsT = kdb[:, blk * TQ + c * P: blk * TQ + (c + 1) * P]
                    else:
                        slot = NBLK * NDIAG + c
                        lhsT = ktb[:, bass.ts(c, P)]
                    s = pss.tile([P, TQ], F32, tag="s")
                    nc.tensor.matmul(s[:], lhsT, qb[0:66, bass.ts(blk, TQ)],
                                     start=True, stop=True)
                    e = work.tile([P, TQ], BF16, tag="e")
                    nc.scalar.activation(e[:], s[:], EXPF, scale=SCALE)
                    if kind == "d":
                        off = 384 - 128 * c
                        nc.vector.tensor_mul(e[:], e[:], wrest[:, off:off + TQ])
                    es.append((e, slot))

                def pv(i):
                    blk, kind, c, st_, sp = chunks[i]
                    e, slot = es[i]
                    nc.tensor.matmul(pvs_ps[blk][0:H + 1, :], vaug[:, slot, :],
                                     e[:], start=st_, stop=sp)
                    if fillers:
                        fillers.pop(0)()

                nxt = 0
                for i in range(len(chunks)):
                    scores(i)
                    if i >= 1 and vts:
                        # v-transposes ride after the early scores (vdh/vfu
                        # copies land later than qb/kdb/ktb)
                        for slot, srcu, col0 in (vts[:2] if i == 1 else vts):
                            make_vaug(slot, srcu, col0)
                        vts = [] if i > 1 else vts[2:]
                    if i >= pipe - 1:
                        pv(nxt)
                        nxt += 1
                while nxt < len(chunks):
                    pv(nxt)
                    nxt += 1
                for f in fillers:
                    f()

            def epilogue_copies(blk):
                pvs = epi.tile([H + 1, TQ], BF16, tag=f"pvs{blk}")
                for j in range(NDIAG):
                    c = bass.ts(j, P)
                    nc.vector.tensor_copy(pvs[:, c], pvs_ps[blk][0:H + 1, c])
                return pvs

            def epi_tr_step(blk, pvs, j):
                def go():
                    ot = pstr.tile([P, H + 1], BF16, tag="tr")
                    nc.tensor.transpose(ot[:], pvs[:, bass.ts(j, P)],
                                        idb_65)
                    r = epi.tile([P, 1], F32, tag="r")
                    nc.vector.reciprocal(r[:], ot[:, H:H + 1])
                    ob = bass.ts(blk * NDIAG + j, H)
                    obt = epi.tile([P, H], BF16, tag="ob")
                    nc.vector.tensor_scalar_mul(obt[:], ot[:, 0:H], r[:])
                    nc.sync.dma_start(out=out_d[:, ob], in_=obt[:])
                return go

            # ---- master schedule ----
            diag = lambda blk: [(blk, "d", c, c == 0, False) for c in range(NDIAG)]
            S1 = proj_xq_steps(1)
            S2 = proj_xk_steps(0)
            S3 = proj_xk_steps(1)
            S4 = proj_xk_steps(2)

            for step in proj_xq_steps(0):
                step()
            pv0 = pspv.tile([H + 1, TQ], F32, tag="pv")
            pvs_ps[0] = pv0
            attn_phase(diag(0), [(d, vdh, d * P) for d in range(NDIAG)],
                       fillers=S1[:4])
            for step in S1[4:]:
                step()
            pv1 = pspv.tile([H + 1, TQ], F32, tag="pv")
            pvs_ps[1] = pv1
            attn_phase(diag(1), [(NDIAG + d, vdh, TQ + d * P) for d in range(NDIAG)],
                       fillers=S2)

            phA = ([(0, "f", c, False, c == 3) for c in range(4)] +
                   [(1, "f", c, False, False) for c in range(4)])
            attn_phase(phA, [(NBLK * NDIAG + c, vfu, c * P) for c in range(4)],
                       fillers=S3)
            pvs0 = epilogue_copies(0)

            attn_phase([(1, "f", c, False, False) for c in range(4, 8)],
                       [(NBLK * NDIAG + c, vfu, c * P) for c in range(4, 8)],
                       fillers=S4)

            attn_phase([(1, "f", c, False, c == 11) for c in range(8, 12)],
                       [(NBLK * NDIAG + c, vfu, c * P) for c in range(8, 12)],
                       fillers=[epi_tr_step(0, pvs0, j) for j in range(NDIAG)])

            pvs1 = epilogue_copies(1)
            for j in range(NDIAG):
                epi_tr_step(1, pvs1, j)()
    nc.compile()
    return nc


def _pack_x(xT, cols):
    # xT: [C, T] fp32 -> [P, CCH*W] bf16 in SBUF layout
    a = xT[:, cols]                                   # [C, W]
    a = a.reshape(CCH, P, -1).transpose(1, 0, 2)      # [P, CCH, W]
    return np.ascontiguousarray(a.reshape(P, -1)).astype(NPBF)


def _pack_w(w):
    # w: [C, width] -> [P, CCH*width]
    a = w.reshape(CCH, P, -1).transpose(1, 0, 2)
    return np.ascontiguousarray(a.reshape(P, -1)).astype(NPBF)


def _host_inputs(x, Wk, Wq, Wv):
    ii = np.arange(P)
    stair = (np.arange(896)[None, :] >= ii[:, None] + 384).astype(NPBF)
    wqk = np.concatenate([
        _pack_w(np.concatenate([Wq, Wv], axis=1)), _pack_w(Wk)], axis=1)
    wrest = np.concatenate([
        stair, _pack_w(np.concatenate([Wk, Wv], axis=1)),
        np.eye(P, dtype=NPBF)], axis=1)
    assert wqk.shape == (P, 1536) and wrest.shape == (P, 2048)
    in_maps = []
    for b in range(B):
        xT = np.ascontiguousarray(x[b].T.astype(np.float32))  # [C, T]
        for h in range(2):
            q0s = (0, 1024) if h == 0 else (512, 1536)
            xq = np.stack([_pack_x(xT, slice(q0, q0 + TQ)) for q0 in q0s])
            xk = np.stack([_pack_x(xT, slice(i * TQ, (i + 1) * TQ))
                           for i in range(NKCH)])
            # ktb bias rows: row blk, col t = 0 if chunk t//128 is a (strictly
            # pre-diagonal) causal chunk for this core's block blk, else BIGNEG
            kaug = np.full((2, KFULL), BIGNEG, np.float32)
            for blk, q0 in enumerate(q0s):
                kaug[blk, :q0] = 0.0
            aug = kaug.astype(NPBF)
            in_maps.append(dict(xq=xq, xk=xk, wqk=wqk, wrest=wrest, aug=aug))
    return in_maps


def kernel(x, Wk, Wq, Wv, trace=False):
    x = np.asarray(x, np.float32)
    in_maps = _host_inputs(x, np.asarray(Wk, np.float32),
                           np.asarray(Wq, np.float32), np.asarray(Wv, np.float32))
    if "nc" not in _CACHE:
        _CACHE["nc"] = build()
    nc = _CACHE["nc"]
    res = run_bass_kernel_spmd(nc, in_maps, list(range(8)), trace=trace)
    out = np.empty((B, T, H), np.float32)
    for b in range(B):
        for h in range(2):
            o = res.results[b * 2 + h]["out"]  # [P, NBLK*NDIAG*H] bf16
            o = np.asarray(o).astype(np.float32).reshape(P, NBLK, NDIAG, H)
            q0s = (0, 1024) if h == 0 else (512, 1536)
            for blk, q0 in enumerate(q0s):
                # row q0 + j*128 + p  <-  o[p, blk, j, :]
                out[b, q0:q0 + TQ] = o[:, blk].transpose(1, 0, 2).reshape(TQ, H)
    kernel.last_exec_time_ns = res.exec_time_ns
    kernel.last_results = res
    return out
